# revision 1
# baseline (speedup 1.0000x reference)
"""Trainium2 Bass kernel for ComplexDFT256.

Math: out[b, 0:256]   = x_real @ cos.T - x_imag @ sin.T
      out[b, 256:512] = x_imag @ cos.T + x_real @ sin.T
which is a single fused matmul  out[B,512] = Z[B,512] @ M[512,512]
with Z = [x_real | x_imag] and M = [[cos.T, sin.T], [-sin.T, cos.T]].

Sharding: pure data parallel over batch across 8 NeuronCores (8192 rows
each). Host pre-transposes Z to [512, B] so the contraction dim lands on
SBUF partitions with perfectly contiguous DMA, and pre-rounds operands
to fp32r (fp32 with 11-bit mantissa; PE runs fp32r at full rate vs 4x
slower for fp32). PSUM accumulates in full fp32. Measured end-to-end
error vs fp64 reference ~1.6e-4 of output scale.
"""
import numpy as np

import concourse.bacc as bacc
import concourse.mybir as mybir
import concourse.tile as tile
from concourse.bass_utils import run_bass_kernel_spmd

N_CORES = 8
BATCH = 65536
FFT = 256
C = 2 * FFT            # contraction dim = 512
J = 2 * FFT            # output features = 512
B_SHARD = BATCH // N_CORES   # 8192
CHUNK_B = 1024         # batch rows loaded per DMA chunk
N_CHUNKS = B_SHARD // CHUNK_B
SUB_PER_CHUNK = CHUNK_B // 128
N_K = C // 128         # 4 contraction tiles

_cache = {}


def _round_fp32r(a: np.ndarray) -> np.ndarray:
    """Round fp32 to fp32r (11-bit mantissa, round-nearest-even).

    Matches neuronxcc static_cast_fp32_to_fp32r bit-exactly; required
    because the BIR verifier insists fp32r matmul inputs are pre-rounded.
    """
    bits = np.ascontiguousarray(a).view(np.uint32)
    lsb = (bits >> 12) & 1
    out = ((bits.astype(np.uint64) + 0x7FF + lsb) & 0xFFFFF000).astype(np.uint32)
    return out.view(np.float32).reshape(a.shape)


def _build_nc(reps: int = 1):
    nc = bacc.Bacc("TRN2", target_bir_lowering=False, debug=False,
                   num_devices=N_CORES)
    f32 = mybir.dt.float32
    f32r = mybir.dt.float32r

    zt_dram = nc.dram_tensor("zt", [C, B_SHARD], f32r, kind="ExternalInput")
    m_dram = nc.dram_tensor("m", [C, J], f32r, kind="ExternalInput")
    out_dram = nc.dram_tensor("out", [B_SHARD, J], f32, kind="ExternalOutput")

    with tile.TileContext(nc) as tc:
        with (
            tc.tile_pool(name="mpool", bufs=1) as mpool,
            tc.tile_pool(name="zpool", bufs=4) as zpool,
            tc.tile_pool(name="opool", bufs=8) as opool,
            tc.tile_pool(name="psum", bufs=6, space="PSUM") as psum_pool,
        ):
            m_sb = []
            for k in range(N_K):
                mt = mpool.tile([128, J], f32r, tag=f"m{k}")
                # SWDGE: keeps the m loads off the SP queue so the first
                # zt chunk streams in parallel
                nc.gpsimd.dma_start(mt[:], m_dram[k * 128:(k + 1) * 128, :])
                m_sb.append(mt)

            def body():
                for i in range(N_CHUNKS):
                    zt_sb = zpool.tile([128, N_K, CHUNK_B], f32r, tag="zt")
                    for k in range(N_K):
                        nc.sync.dma_start(
                            zt_sb[:, k, :],
                            zt_dram[k * 128:(k + 1) * 128,
                                    i * CHUNK_B:(i + 1) * CHUNK_B],
                        )
                    for j in range(SUB_PER_CHUNK):
                        acc = psum_pool.tile([128, J], f32, tag="acc")
                        for k in range(N_K):
                            nc.tensor.matmul(
                                acc[:],
                                zt_sb[:, k, j * 128:(j + 1) * 128],
                                m_sb[k][:],
                                start=(k == 0), stop=(k == N_K - 1),
                            )
                        out_sb = opool.tile([128, J], f32, tag="out")
                        t = i * SUB_PER_CHUNK + j
                        # copies on DVE only; stores issue from the ACT HWDGE
                        # queue so they never head-of-line-block the SP loads
                        nc.vector.tensor_copy(out_sb[:], acc[:])
                        nc.scalar.dma_start(
                            out_dram[t * 128:(t + 1) * 128, :], out_sb[:])

            if reps == 1:
                body()
            else:
                with tc.For_i(0, reps, 1):
                    body()

    nc.compile()
    return nc


def _get_nc():
    if "nc" not in _cache:
        _cache["nc"] = _build_nc()
    return _cache["nc"]


def _prepare_in_maps(x, cos_kernel, sin_kernel):
    x = np.asarray(x, dtype=np.float32)
    cos = np.asarray(cos_kernel, dtype=np.float32)
    sin = np.asarray(sin_kernel, dtype=np.float32)

    m = np.empty((C, J), dtype=np.float32)
    m[:FFT, :FFT] = cos.T
    m[:FFT, FFT:] = sin.T
    m[FFT:, :FFT] = -sin.T
    m[FFT:, FFT:] = cos.T
    m_r = _round_fp32r(m)

    z = _round_fp32r(x.reshape(BATCH, C))
    in_maps = []
    for c in range(N_CORES):
        shard = np.ascontiguousarray(
            z[c * B_SHARD:(c + 1) * B_SHARD, :].T)  # [C, B_SHARD]
        in_maps.append({"zt": shard, "m": m_r})
    return in_maps


def _run(in_maps, trace=False):
    nc = _get_nc()
    return run_bass_kernel_spmd(nc, in_maps, list(range(N_CORES)), trace=trace)


def kernel(x, cos_kernel, sin_kernel):
    in_maps = _prepare_in_maps(x, cos_kernel, sin_kernel)
    res = _run(in_maps)
    out = np.concatenate([r["out"] for r in res.results], axis=0)
    return out.reshape(BATCH, J, 1)



# revision 3
# speedup vs baseline: 1.2988x; 1.2988x over previous
"""Trainium2 Bass kernel for ComplexDFT256.

Math: out[b, 0:256]   = x_real @ cos.T - x_imag @ sin.T
      out[b, 256:512] = x_imag @ cos.T + x_real @ sin.T
which is a single fused matmul  out[B,512] = Z[B,512] @ M[512,512]
with Z = [x_real | x_imag] and M = [[cos.T, sin.T], [-sin.T, cos.T]].

Sharding: pure data parallel over batch across 8 NeuronCores (8192 rows
each). Host pre-transposes Z to [512, B] so the contraction dim lands on
SBUF partitions with perfectly contiguous DMA.

Precision: operands and the stored output are bf16, halving HBM traffic
(16.75 MB/core vs 33.5 MB in fp32) so the DMA floor (~47 us) sits just
under the PE floor (54.6 us @ 2.4 GHz, 1 row/cycle bf16) — the ridge.
PSUM accumulates in fp32; the host upconverts the bf16 result to fp32.
End-to-end error vs the fp32 reference is ~4e-3 of output scale, well
inside the 2e-2 gate.

Engine budget per core and iteration:
 - PE: 256 matmuls x 512 moving rows = 131072 cyc = 54.6 us  <- bound
 - DMA: 8 MB loads (SP queue) + 8 MB stores (ACT queue) ~ 47-50 us
 - PSUM->SBUF bf16 copies round-robined over DVE/Pool/ACT: ~14 us each
"""
import numpy as np
import ml_dtypes

import concourse.bacc as bacc
import concourse.mybir as mybir
import concourse.tile as tile
from concourse.bass_utils import run_bass_kernel_spmd

N_CORES = 8
BATCH = 65536
FFT = 256
C = 2 * FFT            # contraction dim = 512
J = 2 * FFT            # output features = 512
B_SHARD = BATCH // N_CORES   # 8192
CHUNK_B = 2048         # batch rows loaded per DMA chunk
N_CHUNKS = B_SHARD // CHUNK_B
SUB_PER_CHUNK = CHUNK_B // 128
N_K = C // 128         # 4 contraction tiles
G = 4                  # 128-row blocks batched per output store
N_BLOCKS = B_SHARD // 128

_cache = {}

BF16 = ml_dtypes.bfloat16


def _build_nc(reps: int = 1):
    nc = bacc.Bacc("TRN2", target_bir_lowering=False, debug=False,
                   num_devices=N_CORES)
    f32 = mybir.dt.float32
    bf16 = mybir.dt.bfloat16

    # [512, B_SHARD] viewed as [4, 128, B_SHARD] so one strided DMA per
    # chunk lands all 4 contraction tiles
    zt_dram = nc.dram_tensor("zt", [N_K, 128, B_SHARD], bf16,
                             kind="ExternalInput")
    m_dram = nc.dram_tensor("m", [N_K, 128, J], bf16, kind="ExternalInput")
    # [B_SHARD, 512] viewed as [64, 128, 512] for grouped stores
    out_dram = nc.dram_tensor("out", [N_BLOCKS, 128, J], bf16,
                              kind="ExternalOutput")

    with tile.TileContext(nc) as tc:
        with (
            tc.tile_pool(name="mpool", bufs=1) as mpool,
            tc.tile_pool(name="zpool", bufs=3) as zpool,
            tc.tile_pool(name="opool", bufs=3) as opool,
            tc.tile_pool(name="psum", bufs=6, space="PSUM") as psum_pool,
        ):
            m_sb = []
            for k in range(N_K):
                mt = mpool.tile([128, J], bf16, tag=f"m{k}")
                # SWDGE: keeps the m loads off the SP queue so the first
                # zt chunk streams in parallel
                nc.gpsimd.dma_start(mt[:], m_dram[k, :, :])
                m_sb.append(mt)

            def body():
                blk = 0
                for i in range(N_CHUNKS):
                    zt_sb = zpool.tile([128, N_K, CHUNK_B], bf16, tag="zt")
                    nc.sync.dma_start(
                        zt_sb[:],
                        zt_dram[:, :, i * CHUNK_B:(i + 1) * CHUNK_B]
                        .transpose([1, 0, 2]),
                    )
                    for g in range(SUB_PER_CHUNK // G):
                        out_sb = opool.tile([128, G, J], bf16, tag="out")
                        for j2 in range(G):
                            j = g * G + j2
                            acc = psum_pool.tile([128, J], f32, tag="acc")
                            for k in range(N_K):
                                nc.tensor.matmul(
                                    acc[:],
                                    zt_sb[:, k, j * 128:(j + 1) * 128],
                                    m_sb[k][:],
                                    start=(k == 0), stop=(k == N_K - 1),
                                )
                            # spread PSUM->SBUF (+bf16 downcast) copies
                            # across DVE and ACT (GPSIMD can't read PSUM)
                            if blk % 2 == 0:
                                nc.vector.tensor_copy(
                                    out_sb[:, j2, :], acc[:])
                            else:
                                nc.scalar.copy(out_sb[:, j2, :], acc[:])
                            blk += 1
                        t0 = i * SUB_PER_CHUNK + g * G
                        # stores issue from the ACT HWDGE queue so they
                        # never head-of-line-block the SP loads
                        nc.scalar.dma_start(
                            out_dram[t0:t0 + G, :, :].transpose([1, 0, 2]),
                            out_sb[:])

            if reps == 1:
                body()
            else:
                with tc.For_i(0, reps, 1):
                    body()

    nc.compile()
    return nc


def _get_nc():
    if "nc" not in _cache:
        _cache["nc"] = _build_nc()
    return _cache["nc"]


def _prepare_in_maps(x, cos_kernel, sin_kernel):
    x = np.asarray(x, dtype=np.float32)
    cos = np.asarray(cos_kernel, dtype=np.float32)
    sin = np.asarray(sin_kernel, dtype=np.float32)

    m = np.empty((C, J), dtype=np.float32)
    m[:FFT, :FFT] = cos.T
    m[:FFT, FFT:] = sin.T
    m[FFT:, :FFT] = -sin.T
    m[FFT:, FFT:] = cos.T
    m_b = m.astype(BF16).reshape(N_K, 128, J)

    z = x.reshape(BATCH, C).astype(BF16)
    in_maps = []
    for c in range(N_CORES):
        shard = np.ascontiguousarray(
            z[c * B_SHARD:(c + 1) * B_SHARD, :].T)  # [C, B_SHARD]
        in_maps.append({"zt": shard.reshape(N_K, 128, B_SHARD), "m": m_b})
    return in_maps


def _run(in_maps, trace=False):
    nc = _get_nc()
    return run_bass_kernel_spmd(nc, in_maps, list(range(N_CORES)), trace=trace)


def kernel(x, cos_kernel, sin_kernel):
    in_maps = _prepare_in_maps(x, cos_kernel, sin_kernel)
    res = _run(in_maps)
    out = np.concatenate(
        [r["out"].reshape(B_SHARD, J) for r in res.results], axis=0)
    return out.astype(np.float32).reshape(BATCH, J, 1)


# revision 8
# speedup vs baseline: 1.3315x; 1.0252x over previous
"""Trainium2 Bass kernel for ComplexDFT256.

Math: out[b, 0:256]   = x_real @ cos.T - x_imag @ sin.T
      out[b, 256:512] = x_imag @ cos.T + x_real @ sin.T
which is a single fused matmul  out[B,512] = Z[B,512] @ M[512,512]
with Z = [x_real | x_imag] and M = [[cos.T, sin.T], [-sin.T, cos.T]].

Sharding: pure data parallel over batch across 8 NeuronCores (8192 rows
each). Host pre-transposes Z to [512, B] so the contraction dim lands on
SBUF partitions with perfectly contiguous DMA.

Precision: operands and the stored output are bf16, halving HBM traffic
(16.75 MB/core vs 33.5 MB in fp32) so the DMA floor (~47 us) sits just
under the PE floor (54.6 us @ 2.4 GHz, 1 row/cycle bf16) — the ridge.
PSUM accumulates in fp32; the host upconverts the bf16 result to fp32.
End-to-end error vs the fp32 reference is ~4e-3 of output scale, well
inside the 2e-2 gate.

Engine budget per core and iteration:
 - PE: 256 matmuls x 512 moving rows = 131072 cyc = 54.6 us  <- bound
 - DMA: 8 MB loads (SP queue) + 8 MB stores (ACT queue) ~ 47-50 us
 - PSUM->SBUF bf16 copies round-robined over DVE/Pool/ACT: ~14 us each
"""
import numpy as np
import ml_dtypes

import concourse.bacc as bacc
import concourse.mybir as mybir
import concourse.tile as tile
from concourse.bass_utils import run_bass_kernel_spmd

N_CORES = 8
BATCH = 65536
FFT = 256
C = 2 * FFT            # contraction dim = 512
J = 2 * FFT            # output features = 512
B_SHARD = BATCH // N_CORES   # 8192
CHUNK_B = 512          # batch rows loaded per DMA chunk
N_CHUNKS = B_SHARD // CHUNK_B
SUB_PER_CHUNK = CHUNK_B // 128
N_K = C // 128         # 4 contraction tiles
G = 4                  # 128-row blocks batched per output store
N_BLOCKS = B_SHARD // 128

_cache = {}

BF16 = ml_dtypes.bfloat16


def _build_nc(reps: int = 1, unroll: bool = False):
    nc = bacc.Bacc("TRN2", target_bir_lowering=False, debug=False,
                   num_devices=N_CORES)
    f32 = mybir.dt.float32
    bf16 = mybir.dt.bfloat16

    # [512, B_SHARD] viewed as [4, 128, B_SHARD] so one strided DMA per
    # chunk lands all 4 contraction tiles
    zt_dram = nc.dram_tensor("zt", [N_K, 128, B_SHARD], bf16,
                             kind="ExternalInput")
    m_dram = nc.dram_tensor("m", [N_K, 128, J], bf16, kind="ExternalInput")
    # [B_SHARD, 512] viewed as [64, 128, 512] for grouped stores
    out_dram = nc.dram_tensor("out", [N_BLOCKS, 128, J], bf16,
                              kind="ExternalOutput")

    with tile.TileContext(nc) as tc:
        with (
            tc.tile_pool(name="mpool", bufs=1) as mpool,
            tc.tile_pool(name="zpool", bufs=4) as zpool,
            tc.tile_pool(name="opool", bufs=3) as opool,
            tc.tile_pool(name="psum", bufs=6, space="PSUM") as psum_pool,
        ):
            m_sb = []
            for k in range(N_K):
                mt = mpool.tile([128, J], bf16, tag=f"m{k}")
                # on the SP queue AHEAD of the zt chunk loads: same-queue
                # ordering guarantees all m tiles land before chunk 0, so
                # the PE never stalls mid-stream waiting for weights
                nc.sync.dma_start(mt[:], m_dram[k, :, :])
                m_sb.append(mt)

            def body():
                blk = 0
                for i in range(N_CHUNKS):
                    zt_sb = zpool.tile([128, N_K, CHUNK_B], bf16, tag="zt")
                    nc.sync.dma_start(
                        zt_sb[:],
                        zt_dram[:, :, i * CHUNK_B:(i + 1) * CHUNK_B]
                        .transpose([1, 0, 2]),
                    )
                    for g in range(SUB_PER_CHUNK // G):
                        out_sb = opool.tile([128, G, J], bf16, tag="out")
                        for j2 in range(G):
                            j = g * G + j2
                            acc = psum_pool.tile([128, J], f32, tag="acc")
                            for k in range(N_K):
                                nc.tensor.matmul(
                                    acc[:],
                                    zt_sb[:, k, j * 128:(j + 1) * 128],
                                    m_sb[k][:],
                                    start=(k == 0), stop=(k == N_K - 1),
                                )
                            # spread PSUM->SBUF (+bf16 downcast) copies
                            # across DVE and ACT (GPSIMD can't read PSUM)
                            if blk % 2 == 0:
                                nc.vector.tensor_copy(
                                    out_sb[:, j2, :], acc[:])
                            else:
                                nc.scalar.copy(out_sb[:, j2, :], acc[:])
                            blk += 1
                        t0 = i * SUB_PER_CHUNK + g * G
                        # stores issue from the ACT HWDGE queue so they
                        # never head-of-line-block the SP loads
                        nc.scalar.dma_start(
                            out_dram[t0:t0 + G, :, :].transpose([1, 0, 2]),
                            out_sb[:])

            if reps == 1:
                body()
            elif unroll:
                for _ in range(reps):
                    body()
            else:
                with tc.For_i(0, reps, 1):
                    body()

    nc.compile()
    return nc


def _get_nc():
    if "nc" not in _cache:
        _cache["nc"] = _build_nc()
    return _cache["nc"]


def _prepare_in_maps(x, cos_kernel, sin_kernel):
    x = np.asarray(x, dtype=np.float32)
    cos = np.asarray(cos_kernel, dtype=np.float32)
    sin = np.asarray(sin_kernel, dtype=np.float32)

    m = np.empty((C, J), dtype=np.float32)
    m[:FFT, :FFT] = cos.T
    m[:FFT, FFT:] = sin.T
    m[FFT:, :FFT] = -sin.T
    m[FFT:, FFT:] = cos.T
    m_b = m.astype(BF16).reshape(N_K, 128, J)

    z = x.reshape(BATCH, C).astype(BF16)
    in_maps = []
    for c in range(N_CORES):
        shard = np.ascontiguousarray(
            z[c * B_SHARD:(c + 1) * B_SHARD, :].T)  # [C, B_SHARD]
        in_maps.append({"zt": shard.reshape(N_K, 128, B_SHARD), "m": m_b})
    return in_maps


def _run(in_maps, trace=False):
    nc = _get_nc()
    return run_bass_kernel_spmd(nc, in_maps, list(range(N_CORES)), trace=trace)


def kernel(x, cos_kernel, sin_kernel):
    in_maps = _prepare_in_maps(x, cos_kernel, sin_kernel)
    res = _run(in_maps)
    out = np.concatenate(
        [r["out"].reshape(B_SHARD, J) for r in res.results], axis=0)
    return out.astype(np.float32).reshape(BATCH, J, 1)


# revision 12
# speedup vs baseline: 1.5412x; 1.1575x over previous
"""Trainium2 Bass kernel for ComplexDFT256.

Math: out[b, 0:256]   = x_real @ cos.T - x_imag @ sin.T
      out[b, 256:512] = x_imag @ cos.T + x_real @ sin.T
which is a single fused matmul  out[B,512] = Z[B,512] @ M[512,512]
with Z = [x_real | x_imag] and M = [[cos.T, sin.T], [-sin.T, cos.T]].

Sharding: pure data parallel over batch across 8 NeuronCores (8192 rows
each). Host pre-transposes Z to [512, B] so the contraction dim lands on
SBUF partitions with perfectly contiguous DMA.

Precision: operands and the stored output are bf16, halving HBM traffic
(16.75 MB/core vs 33.5 MB in fp32) so the DMA floor (~47 us) sits just
under the PE floor (54.6 us @ 2.4 GHz, 1 row/cycle bf16) — the ridge.
PSUM accumulates in fp32; the host upconverts the bf16 result to fp32.
End-to-end error vs the fp32 reference is ~4e-3 of output scale, well
inside the 2e-2 gate.

Engine budget per core and iteration:
 - PE: 256 matmuls x 512 moving rows = 131072 cyc = 54.6 us  <- bound
 - DMA: 8 MB loads (SP queue) + 8 MB stores (ACT queue) ~ 47-50 us
 - PSUM->SBUF bf16 copies round-robined over DVE/Pool/ACT: ~14 us each
"""
import numpy as np
import ml_dtypes

import concourse.bacc as bacc
import concourse.mybir as mybir
import concourse.tile as tile
from concourse.bass_utils import run_bass_kernel_spmd

N_CORES = 8
BATCH = 65536
FFT = 256
C = 2 * FFT            # contraction dim = 512
J = 2 * FFT            # output features = 512
B_SHARD = BATCH // N_CORES   # 8192
CHUNK_B = 512          # batch rows loaded per DMA chunk
N_CHUNKS = B_SHARD // CHUNK_B
SUB_PER_CHUNK = CHUNK_B // 128
N_K = C // 128         # 4 contraction tiles
G = 4                  # 128-row blocks batched per output store
N_BLOCKS = B_SHARD // 128

_cache = {}

BF16 = ml_dtypes.bfloat16


def _build_nc(reps: int = 1, unroll: bool = False):
    nc = bacc.Bacc("TRN2", target_bir_lowering=False, debug=False,
                   num_devices=N_CORES)
    f32 = mybir.dt.float32
    bf16 = mybir.dt.bfloat16

    # [512, B_SHARD] viewed as [4, 128, B_SHARD] so one strided DMA per
    # chunk lands all 4 contraction tiles
    zt_dram = nc.dram_tensor("zt", [N_K, 128, B_SHARD], bf16,
                             kind="ExternalInput")
    m_dram = nc.dram_tensor("m", [N_K, 128, J], bf16, kind="ExternalInput")
    # [B_SHARD, 512] viewed as [64, 128, 512] for grouped stores
    out_dram = nc.dram_tensor("out", [N_BLOCKS, 128, J], bf16,
                              kind="ExternalOutput")

    with tile.TileContext(nc) as tc:
        with (
            tc.tile_pool(name="mpool", bufs=1) as mpool,
            tc.tile_pool(name="zpool", bufs=4) as zpool,
            tc.tile_pool(name="opool", bufs=3) as opool,
            tc.tile_pool(name="psum", bufs=6, space="PSUM") as psum_pool,
        ):
            m_sb = []
            for k in range(N_K):
                mt = mpool.tile([128, J], bf16, tag=f"m{k}")
                # on the SP queue AHEAD of the zt chunk loads: same-queue
                # ordering guarantees all m tiles land before chunk 0, so
                # the PE never stalls mid-stream waiting for weights
                nc.sync.dma_start(mt[:], m_dram[k, :, :])
                m_sb.append(mt)

            def body():
                blk = 0
                for i in range(N_CHUNKS):
                    zt_sb = zpool.tile([128, N_K, CHUNK_B], bf16, tag="zt")
                    nc.sync.dma_start(
                        zt_sb[:],
                        zt_dram[:, :, i * CHUNK_B:(i + 1) * CHUNK_B]
                        .transpose([1, 0, 2]),
                    )
                    for g in range(SUB_PER_CHUNK // G):
                        out_sb = opool.tile([128, G, J], bf16, tag="out")
                        for j2 in range(G):
                            j = g * G + j2
                            acc = psum_pool.tile([128, J], f32, tag="acc")
                            for k in range(N_K):
                                nc.tensor.matmul(
                                    acc[:],
                                    zt_sb[:, k, j * 128:(j + 1) * 128],
                                    m_sb[k][:],
                                    start=(k == 0), stop=(k == N_K - 1),
                                )
                            # spread PSUM->SBUF (+bf16 downcast) copies
                            # across DVE and ACT (GPSIMD can't read PSUM)
                            if blk % 2 == 0:
                                nc.vector.tensor_copy(
                                    out_sb[:, j2, :], acc[:])
                            else:
                                nc.scalar.copy(out_sb[:, j2, :], acc[:])
                            blk += 1
                        t0 = i * SUB_PER_CHUNK + g * G
                        # stores issue from the ACT HWDGE queue so they
                        # never head-of-line-block the SP loads
                        nc.scalar.dma_start(
                            out_dram[t0:t0 + G, :, :].transpose([1, 0, 2]),
                            out_sb[:])

            if reps == 1:
                body()
            elif unroll:
                for _ in range(reps):
                    body()
            else:
                # unroll U bodies per hardware-loop iteration: the For_i
                # back-edge carries an all-engine barrier (semaphore
                # reset), so amortize it + the post-barrier pipeline
                # refill over U bodies
                U = 8
                full, rem = divmod(reps, U)
                if full:
                    with tc.For_i(0, full, 1):
                        for _ in range(U):
                            body()
                for _ in range(rem):
                    body()

    nc.compile()
    return nc


def _get_nc():
    if "nc" not in _cache:
        _cache["nc"] = _build_nc()
    return _cache["nc"]


def _prepare_in_maps(x, cos_kernel, sin_kernel):
    x = np.asarray(x, dtype=np.float32)
    cos = np.asarray(cos_kernel, dtype=np.float32)
    sin = np.asarray(sin_kernel, dtype=np.float32)

    m = np.empty((C, J), dtype=np.float32)
    m[:FFT, :FFT] = cos.T
    m[:FFT, FFT:] = sin.T
    m[FFT:, :FFT] = -sin.T
    m[FFT:, FFT:] = cos.T
    m_b = m.astype(BF16).reshape(N_K, 128, J)

    z = x.reshape(BATCH, C).astype(BF16)
    in_maps = []
    for c in range(N_CORES):
        shard = np.ascontiguousarray(
            z[c * B_SHARD:(c + 1) * B_SHARD, :].T)  # [C, B_SHARD]
        in_maps.append({"zt": shard.reshape(N_K, 128, B_SHARD), "m": m_b})
    return in_maps


def _run(in_maps, trace=False):
    nc = _get_nc()
    return run_bass_kernel_spmd(nc, in_maps, list(range(N_CORES)), trace=trace)


def kernel(x, cos_kernel, sin_kernel):
    in_maps = _prepare_in_maps(x, cos_kernel, sin_kernel)
    res = _run(in_maps)
    out = np.concatenate(
        [r["out"].reshape(B_SHARD, J) for r in res.results], axis=0)
    return out.astype(np.float32).reshape(BATCH, J, 1)
